# revision 24
# baseline (speedup 1.0000x reference)
"""Trainium2 Bass kernel for an FFM (field-aware factorization machine) layer.

Reference computation (B=16384, P=512, F=16, K=8):
    A[i,j,:] = v[i, f2f[j], :]
    S[i,j]   = sum_k A[i,j,k] * A[j,i,k]          (symmetric)
    rp[b]    = sum_{i<j} x[b,i] * S[i,j] * x[b,j]
    out      = x @ w + rp[:,None] + b

Because S is symmetric, the strictly-upper-triangular quadratic form reduces to
    rp[b] = x[b] @ M @ x[b]^T,   M = 0.5 * (S - diag(S))
so with y' = x @ M + 1*w^T (a plain [512,512] matmul):
    out[b] = sum_j x[b,j] * (y'[b,j]) + bias

Host side folds (v, f2f, w) -> M (a tiny 512x512x8 einsum, ~0.4% of the FLOPs);
the device does the dominant work: the 16384x512x512 matmul, the elementwise
multiply and both reductions, data-parallel over batch across 8 NeuronCores.

Device kernel (per core, batch shard of 2048 rows), transposed orientation:
    x^T tiles produced on-chip: HBM fp32 load -> DVE cast fp16 -> XBAR DMA
    transpose.  y'^T[j,b] accumulated in PSUM from 4 K=128 fp16 matmuls with
    M-chunks stationary; a fused DVE scalar_tensor_tensor computes
    z = (y'^T + w) * x^T; a ones-vector matmul reduces z over partitions into
    rp^T; ACT adds the scalar bias; result DMAs out.
"""

import os
from contextlib import ExitStack

import numpy as np
import ml_dtypes

import concourse.bass as bass
import concourse.mybir as mybir
import concourse.tile as tile
from concourse import bacc
from concourse.bass import ds, ts
from concourse.bass_utils import run_bass_kernel_spmd

B, P, F, K = 16384, 512, 16, 8
N_CORES = 8
B_SH = B // N_CORES          # 2048 batch rows per core
BT = 512                     # batch tile (free dim of transposed tiles)
NBT = B_SH // BT             # 4 batch tiles per core
NC128 = P // 128             # 4 chunks of 128 along the feature dim

FP32 = mybir.dt.float32
FP16 = mybir.dt.float16

# test.py can read this after calling kernel() (exec_time_ns etc.)
LAST_RESULT = None


def _build_nc(bias: float) -> bass.Bass:
    nc = bacc.Bacc("TRN2", target_bir_lowering=False, debug=False,
                   num_devices=N_CORES)

    x_d = nc.dram_tensor("x", [B_SH, P], FP32, kind="ExternalInput")
    # m_d[p, c, j] = M[c*128 + p, j]  (fp16, host-prepared)
    m_d = nc.dram_tensor("m", [128, NC128, P], FP16, kind="ExternalInput")
    # w_d[p, c] = w[c*128 + p]
    w_d = nc.dram_tensor("w", [128, NC128], FP32, kind="ExternalInput")
    out_d = nc.dram_tensor("out", [B_SH, 1], FP32, kind="ExternalOutput")

    with tile.TileContext(nc) as tc, ExitStack() as ctx:
        const = ctx.enter_context(tc.tile_pool(name="const", bufs=1))
        xn16p = ctx.enter_context(tc.tile_pool(name="xn16", bufs=3))
        xtp = ctx.enter_context(tc.tile_pool(name="xt", bufs=2))
        zp = ctx.enter_context(tc.tile_pool(name="z", bufs=3))
        orp = ctx.enter_context(tc.tile_pool(name="orow", bufs=2))
        pyp = ctx.enter_context(tc.tile_pool(name="py", bufs=2, space="PSUM"))
        prp = ctx.enter_context(tc.tile_pool(name="pr", bufs=1, space="PSUM"))
        pxp = ctx.enter_context(tc.tile_pool(name="px", bufs=2, space="PSUM"))

        ident = const.tile([128, 128], FP16)
        from concourse.masks import make_identity
        make_identity(nc, ident[:])

        mt = const.tile([128, NC128, P], FP16)
        wt = const.tile([128, NC128], FP32)
        ones = const.tile([128, 1], FP16)
        nc.vector.memset(ones[:], 1.0)

        # natural-layout batch tiles: [p, bn, i], row = bt*512 + bn*128 + p
        x_tiles = x_d.ap().rearrange("(t bn p) i -> t p bn i", p=128, bn=BT // 128)
        out_rows = out_d.ap().rearrange("(t b) one -> t one b", t=NBT)

        # HAM warmup: keep the PE busy through the initial DMA window so the
        # first real transposes/matmuls run closer to 2.4 GHz.
        wps = pyp.tile([128, 512], FP32, tag="py")
        for _ in range(14):
            nc.tensor.matmul(wps[:, :128], lhsT=ident[:], rhs=ident[:],
                             start=True, stop=True)

        for bt in range(NBT):
            # ---- x^T fp16 tiles via PE transposes; fp32->fp16 cast in-DMA.
            # Per-slab loads and per-slab PSUM->SBUF copies keep the
            # load -> transpose -> copy -> matmul chain finely pipelined.
            xn16 = xn16p.tile([128, BT // 128, P], FP16)
            px = pxp.tile([128, NC128, BT], FP16)
            xt = xtp.tile([128, NC128, BT], FP16)
            for bn in range(BT // 128):
                nc.gpsimd.dma_start(xn16[:, bn, :], x_tiles[bt, :, bn, :])
                if bt == 0 and bn == 0:
                    nc.gpsimd.dma_start(mt[:], m_d.ap())
                    nc.gpsimd.dma_start(wt[:], w_d.ap())
                for ic in range(NC128):
                    nc.tensor.transpose(px[:, ic, ds(bn * 128, 128)],
                                        xn16[:, bn, ts(ic, 128)], ident[:])
                nc.vector.tensor_copy(xt[:, :, ds(bn * 128, 128)],
                                      px[:, :, ds(bn * 128, 128)])

            # ---- y'^T = M^T-chunks @ x^T ; z = (y'^T + w) * x^T ; reduce ----
            pr = prp.tile([1, BT], FP32)
            for jc in range(NC128):
                py = pyp.tile([128, BT], FP32)
                for ic in range(NC128):
                    nc.tensor.matmul(py[:], lhsT=mt[:, ic, ts(jc, 128)],
                                     rhs=xt[:, ic, :],
                                     start=(ic == 0), stop=(ic == NC128 - 1))
                z = zp.tile([128, BT], FP16)
                nc.vector.scalar_tensor_tensor(
                    out=z[:], in0=py[:], scalar=wt[:, jc:jc + 1],
                    in1=xt[:, jc, :],
                    op0=mybir.AluOpType.add, op1=mybir.AluOpType.mult)
                nc.tensor.matmul(pr[:], lhsT=ones[:], rhs=z[:],
                                 start=(jc == 0), stop=(jc == NC128 - 1))

            orow = orp.tile([1, BT], FP32)
            nc.scalar.activation(orow[:], pr[:],
                                 mybir.ActivationFunctionType.Copy,
                                 bias=float(bias), scale=1.0)
            nc.gpsimd.dma_start(out_rows[bt], orow[:])

    nc.compile()
    return nc


def kernel(x: np.ndarray, w: np.ndarray, v: np.ndarray, b: np.ndarray,
           f2f: np.ndarray) -> np.ndarray:
    global LAST_RESULT
    x = np.ascontiguousarray(np.asarray(x, dtype=np.float32))
    w = np.asarray(w, dtype=np.float32)
    v = np.asarray(v, dtype=np.float32)
    b = np.asarray(b, dtype=np.float32)
    f2f = np.asarray(f2f, dtype=np.int32)

    # ---- host: fold (v, f2f) into the interaction matrix M ----
    A = v[:, f2f, :]                                # [P, P, K]
    S = np.einsum('ijk,jik->ij', A, A)              # [P, P], symmetric
    M = 0.5 * (S - np.diag(np.diag(S)))             # strict-triu quadratic form

    m_host = np.ascontiguousarray(
        M.reshape(NC128, 128, P).transpose(1, 0, 2).astype(np.float16))
    # [128, NC128, P] fp16: m_host[p, c, j] = M[c*128 + p, j]
    w_host = np.ascontiguousarray(
        w[:, 0].reshape(NC128, 128).T.astype(np.float32))  # [128, NC128]
    bias = float(b[0])

    nc = _build_nc(bias)

    in_maps = []
    for c in range(N_CORES):
        in_maps.append({
            "x": np.ascontiguousarray(x[c * B_SH:(c + 1) * B_SH]),
            "m": m_host,
            "w": w_host,
        })

    res = run_bass_kernel_spmd(nc, in_maps, core_ids=list(range(N_CORES)))
    LAST_RESULT = res

    out = np.concatenate([r["out"] for r in res.results], axis=0)
    return out.astype(np.float32)


if __name__ == "__main__":
    rng = np.random.default_rng(0)
    xs = rng.standard_normal((B, P), dtype=np.float32)
    ws = (rng.standard_normal((P, 1)) * 0.05).astype(np.float32)
    vs = (rng.standard_normal((P, F, K)) * 0.05).astype(np.float32)
    bs = rng.standard_normal((1,)).astype(np.float32)
    fs = rng.integers(0, F, size=(P,)).astype(np.int32)
    o = kernel(x=xs, w=ws, v=vs, b=bs, f2f=fs)
    print("out", o.shape, o.dtype, o[:4, 0])
